# revision 21
# baseline (speedup 1.0000x reference)
"""BitFeedForward (Hadamard + int8 act-quant + ternary weights) on 8 TRN2 cores.

v3 — data-parallel over tokens (8192 -> 1024/core), restructured from v2:
  - weight ternarization sharded 8x across cores (each core ternarizes 1/8
    of w_up and w_down), then fp8 AllGather; t1 resident in SBUF, t2
    streamed once per block from the gathered DRAM buffer.
  - per-token absmax via PE-transpose + free-dim reduce + K=1 matmul
    broadcast (no gpsimd partition_all_reduce on the hot path).
  - plain H128 matmul stages (single stationary operand) + in-place
    DVE/GPSIMD butterflies for all inter-chunk FWHT stages.
  - relu^2 evacuation fused into one DVE scalar_tensor_tensor per slice.
  - 2-deep software-pipelined emission so the PE stream
    [gemm2(k-2) | xA(k+1) | gemm1(k) | h128_2(k)] never waits on the DVE
    butterfly chain.
"""
import math
import numpy as np
from contextlib import ExitStack

import concourse.bass as bass
from concourse import bacc
import concourse.tile as tile
import concourse.mybir as mybir
from concourse.bass_utils import run_bass_kernel_spmd
from concourse.masks import make_identity

F32 = mybir.dt.float32
BF16 = mybir.dt.bfloat16
FP8 = mybir.dt.float8e4

NCORES = 8
B, S, H, I = 4, 2048, 2048, 4096
TOKENS = B * S
T = TOKENS // NCORES
TB = 256
NB = T // TB
TH = 128
NC1 = H // 128
NC2 = I // 128
C_MAGIC = 12582912.0
ISQ1 = 1.0 / math.sqrt(H)
WCOUNT = float(H * I)
FR1 = 0.82  # DVE share of FWHT1 butterflies
FR2 = 0.80  # DVE share of FWHT2 butterflies
MSF = 0.62  # DVE share of the scale-multiplies

ADD = mybir.AluOpType.add
SUB = mybir.AluOpType.subtract
MULT = mybir.AluOpType.mult
MAX = mybir.AluOpType.max
MIN = mybir.AluOpType.min
BYPASS = mybir.AluOpType.bypass
AF = mybir.ActivationFunctionType
AX = mybir.AxisListType.X


def _bfly_ip(t, nchunk, sigma, span):
    """In-place butterfly over the chunk axis on DVE: a' = a+b ; b' = a'-2b."""
    v = t[:].rearrange("p (g two s) -> p g two s", two=2, s=sigma * span)
    a = v[:, :, 0, :]
    bv = v[:, :, 1, :]
    nc_v = _bfly_ip.nc.vector
    nc_v.tensor_tensor(a, a, bv, ADD)
    nc_v.scalar_tensor_tensor(bv, bv, -2.0, a, MULT, ADD)


def build():
    nc = bacc.Bacc()
    _bfly_ip.nc = nc
    x_in = nc.declare_dram_parameter("x", [T, H], F32, isOutput=False)
    wus_in = nc.declare_dram_parameter("wu_s", [H // NCORES, I], F32, isOutput=False)
    wds_in = nc.declare_dram_parameter("wd_s", [I // NCORES, H], F32, isOutput=False)
    h128_in = nc.declare_dram_parameter("h128", [128, 128], F32, isOutput=False)
    out_d = nc.declare_dram_parameter("out", [T, H], F32, isOutput=True)

    RG = [list(range(NCORES))]

    with tile.TileContext(nc) as tc, ExitStack() as ctx:
        const = ctx.enter_context(tc.tile_pool(name="const", bufs=1))
        wq = ctx.enter_context(tc.tile_pool(name="wq", bufs=2))
        tsb = ctx.enter_context(tc.tile_pool(name="tsb", bufs=2))
        t1p = ctx.enter_context(tc.tile_pool(name="t1", bufs=1))
        slab = ctx.enter_context(tc.tile_pool(name="slab", bufs=3))
        xinp = ctx.enter_context(tc.tile_pool(name="xinp", bufs=3))
        fw1 = ctx.enter_context(tc.tile_pool(name="fw1", bufs=2))
        q1p = ctx.enter_context(tc.tile_pool(name="q1", bufs=3))
        rp = ctx.enter_context(tc.tile_pool(name="rp", bufs=2))
        q2p = ctx.enter_context(tc.tile_pool(name="q2", bufs=2))
        otp = ctx.enter_context(tc.tile_pool(name="otp", bufs=2))
        scl = ctx.enter_context(tc.tile_pool(name="scl", bufs=2))
        tiny = ctx.enter_context(tc.tile_pool(name="tiny", bufs=6))
        dram = ctx.enter_context(tc.tile_pool(name="dram", bufs=1, space="DRAM"))
        ps_m = ctx.enter_context(tc.tile_pool(name="ps_m", bufs=2, space="PSUM"))
        ps_g1 = ctx.enter_context(tc.tile_pool(name="ps_g1", bufs=2, space="PSUM"))
        ps_g2 = ctx.enter_context(tc.tile_pool(name="ps_g2", bufs=4, space="PSUM"))

        ident = const.tile([128, 128], F32)
        make_identity(nc, ident[:])
        h128 = const.tile([128, 128], F32)
        nc.sync.dma_start(h128[:], h128_in[:])
        ones_col = const.tile([128, 1], F32)
        nc.vector.memset(ones_col[:], 1.0)
        ones_row = const.tile([1, 128], F32)
        nc.vector.memset(ones_row[:], 1.0)
        biasC = const.tile([128, 1], F32)
        nc.vector.memset(biasC[:], C_MAGIC)
        biasNC = const.tile([128, 1], F32)
        nc.vector.memset(biasNC[:], -C_MAGIC)

        # ------------- weight scale partials + AllReduce -------------
        def wchunks(src, nrow, width):
            """Yield ([128,2048] slice-view, row, colhalf) subchunks."""
            for i in range(nrow):
                for hc in range(width // 2048):
                    yield (src[i * 128:(i + 1) * 128,
                               hc * 2048:(hc + 1) * 2048], i, hc)

        def abs_total(src, nrow, width, tagp):
            parts = []
            for sl, i, hc in wchunks(src, nrow, width):
                ch = wq.tile([128, 2048], F32, tag="wch", name=f"w_{tagp}_{i}_{hc}")
                nc.gpsimd.dma_start(ch[:], sl)
                acc = tiny.tile([128, 1], F32, tag="wacc")
                nc.vector.tensor_reduce(acc[:], ch[:], AX, ADD,
                                        apply_absolute_value=True)
                parts.append(acc)
            tot = tiny.tile([128, 1], F32, tag=f"{tagp}tot")
            nc.vector.tensor_add(tot[:], parts[0][:], parts[1][:])
            for a in parts[2:]:
                nc.vector.tensor_add(tot[:], tot[:], a[:])
            return tot

        pu = abs_total(wus_in, 2, I, "au")
        pd = abs_total(wds_in, 4, H, "ad")
        psum2 = ps_m.tile([1, 2], F32, tag="pm")
        nc.tensor.matmul(psum2[:, 0:1], pu[:], ones_col[:], start=True, stop=True)
        nc.tensor.matmul(psum2[:, 1:2], pd[:], ones_col[:], start=True, stop=True)
        part = tiny.tile([1, 2], F32)
        nc.vector.tensor_copy(part[:], psum2[:])

        ccin = dram.tile([1, 2], F32, tag="ccin")
        ccg = dram.tile([NCORES, 2], F32, tag="ccg", addr_space="Shared")
        nc.gpsimd.dma_start(ccin[:], part[:])
        nc.gpsimd.collective_compute(
            "AllGather", BYPASS, replica_groups=RG,
            ins=[ccin.opt()], outs=[ccg.opt()])

        ones8 = const.tile([8, 128], F32)
        nc.vector.memset(ones8[:], 1.0)

        def finish_scales():
            cc8 = tiny.tile([8, 2], F32)
            nc.gpsimd.dma_start(cc8[:], ccg[:])
            # sum the 8 rank rows AND broadcast to 128 partitions in one mm
            sW_ps = ps_m.tile([128, 2], F32, tag="pm")
            nc.tensor.matmul(sW_ps[:], ones8[:], cc8[:], start=True, stop=True)
            sW = tiny.tile([128, 2], F32)
            nc.vector.tensor_copy(sW[:], sW_ps[:])
            nc.vector.tensor_scalar(sW[:], sW[:], 1.0 / WCOUNT, 1e-5, MULT, MAX)
            rW = tiny.tile([128, 2], F32)
            nc.vector.reciprocal(rW[:], sW[:])
            return sW, rW

        # ------------- sharded ternarize + fp8 AllGathers (t1 first) -------------
        t1sA = dram.tile([H // NCORES, I // 2], FP8, tag="t1sA")
        t1sB = dram.tile([H // NCORES, I // 2], FP8, tag="t1sB")
        t1gA = dram.tile([H, I // 2], FP8, tag="t1gA", addr_space="Shared")
        t1gB = dram.tile([H, I // 2], FP8, tag="t1gB", addr_space="Shared")
        t2s = dram.tile([I // NCORES, H], FP8, tag="t2s")
        t2g = dram.tile([I, H], FP8, tag="t2g", addr_space="Shared")

        def ternarize(rW, src, nrow, width, col, tagp, dst_of):
            for sidx, (sl, i, hc) in enumerate(wchunks(src, nrow, width)):
                ch = wq.tile([128, 2048], F32, tag="wch", name=f"wb_{tagp}_{i}_{hc}")
                nc.sync.dma_start(ch[:], sl)
                ob = tsb.tile([128, 2048], FP8, tag="tsb")
                for q in range(4):
                    tmp = ps_m.tile([128, 512], F32, tag="pm")
                    nc.scalar.activation(tmp[:], ch[:, q * 512:(q + 1) * 512],
                                         AF.Identity, bias=biasC[:],
                                         scale=rW[:, col:col + 1])
                    nc.scalar.activation(ob[:, q * 512:(q + 1) * 512], tmp[:],
                                         AF.Sign, bias=biasNC[:], scale=1.0)
                nc.sync.dma_start(dst_of(i, hc), ob[:])

        def t1_dst(i, hc):
            g = t1sA if hc == 0 else t1sB
            return g[i * 128:(i + 1) * 128, :]

        def t2_dst(i, hc):
            return t2s[i * 128:(i + 1) * 128, :]

        def weights_phase2(sW, rW):
            # t1 I-column half A first: its AllGather (small -> mesh algo)
            # unblocks the first half of GEMM1's output chunks early.
            for hc, shard, gath in ((0, t1sA, t1gA), (1, t1sB, t1gB)):
                for i in range(2):
                    sl = wus_in[i * 128:(i + 1) * 128,
                                hc * 2048:(hc + 1) * 2048]
                    ch = wq.tile([128, 2048], F32, tag="wch",
                                 name=f"wb_u_{i}_{hc}")
                    nc.sync.dma_start(ch[:], sl)
                    ob = tsb.tile([128, 2048], FP8, tag="tsb")
                    for q in range(4):
                        tmp = ps_m.tile([128, 512], F32, tag="pm")
                        nc.scalar.activation(tmp[:], ch[:, q * 512:(q + 1) * 512],
                                             AF.Identity, bias=biasC[:],
                                             scale=rW[:, 0:1])
                        nc.scalar.activation(ob[:, q * 512:(q + 1) * 512],
                                             tmp[:], AF.Sign, bias=biasNC[:],
                                             scale=1.0)
                    nc.sync.dma_start(shard[i * 128:(i + 1) * 128, :], ob[:])
                nc.gpsimd.collective_compute(
                    "AllGather", BYPASS, replica_groups=RG,
                    ins=[shard.opt()], outs=[gath.opt()])
            ternarize(rW, wds_in, 4, H, 1, "d", t2_dst)
            nc.gpsimd.collective_compute(
                "AllGather", BYPASS, replica_groups=RG,
                ins=[t2s.opt()], outs=[t2g.opt()])
            t1 = t1p.tile([128, NC1 * I], FP8)  # resident ternary w_up^T
            for kc in range(NC1):
                nc.sync.dma_start(t1[:, kc * I:kc * I + 2048],
                                  t1gA[kc * 128:(kc + 1) * 128, :])
            for kc in range(NC1):
                nc.sync.dma_start(t1[:, kc * I + 2048:(kc + 1) * I],
                                  t1gB[kc * 128:(kc + 1) * 128, :])
            return t1

        # ------------- per-token scale helpers -------------
        def col_scales_bcast(vec):
            """[128,1] token-on-partition vector -> [128, TH] SBUF broadcast."""
            rps = ps_m.tile([1, 128], F32, tag="pm")
            nc.tensor.transpose(rps[:], vec[:], ident[:])
            row = tiny.tile([1, 128], F32, tag="row")
            nc.vector.tensor_copy(row[:], rps[:])
            bps = ps_m.tile([128, 128], F32, tag="pm")
            nc.tensor.matmul(bps[:], ones_row[:], row[:], start=True, stop=True)
            sb = scl.tile([128, TH], F32, tag="bc")
            nc.scalar.copy(sb[:], bps[:])
            return sb

        def x_half(b, j, q1):
            """Transpose + FWHT + int8 quant for token-half j of block b.

            Returns the per-token M1 vector (kept for layer-2 scales)."""
            tok0 = b * TB + j * TH
            u = fw1.tile([128, NC1 * TH], F32, tag="fw1")
            for g in range(4):
                xs = xinp.tile([128, 512], F32, tag="xin")
                nc.sync.dma_start(
                    xs[:], x_in[tok0:tok0 + TH, g * 512:(g + 1) * 512])
                pt = ps_m.tile([128, 512], F32, tag="pm")
                for k in range(4):
                    nc.tensor.transpose(
                        pt[:, k * 128:(k + 1) * 128],
                        xs[:, k * 128:(k + 1) * 128], ident[:])
                nc.scalar.copy(u[:, g * 512:(g + 1) * 512], pt[:])
            for g in range(4):
                u1 = ps_m.tile([128, 512], F32, tag="pm")
                for k in range(4):
                    c = 4 * g + k
                    nc.tensor.matmul(u1[:, k * TH:(k + 1) * TH], h128[:],
                                     u[:, c * TH:(c + 1) * TH],
                                     start=True, stop=True)
                nc.scalar.copy(u[:, g * 512:(g + 1) * 512], u1[:])
            for sg in (1, 2, 4, 8):
                _bfly_ip(u, NC1, sg, TH)
            # per-token absmax over [partitions x chunks]
            P1 = scl.tile([128, TH], F32, tag="p1")
            nc.vector.tensor_reduce(
                P1[:], u[:].rearrange("p (c t) -> p t c", c=NC1),
                AX, MAX, apply_absolute_value=True)
            tps = ps_m.tile([128, 128], F32, tag="pm")
            nc.tensor.transpose(tps[:], P1[:], ident[:])
            M1 = tiny.tile([128, 1], F32, tag="m1")
            nc.vector.tensor_reduce(M1[:], tps[:], AX, MAX,
                                    apply_absolute_value=True)
            nc.vector.tensor_scalar(M1[:], M1[:], ISQ1, 1e-5, MULT, MAX)
            s1t = tiny.tile([128, 1], F32, tag="s1t")
            nc.vector.reciprocal(s1t[:], M1[:])
            nc.vector.tensor_scalar(s1t[:], s1t[:], 127.0 * ISQ1, None, MULT)
            s1b = col_scales_bcast(s1t)
            uv = u[:].rearrange("p (c t) -> p c t", c=NC1)
            sbb = s1b[:, None, :].broadcast_to([128, NC1, TH])
            MS = int(NC1 * MSF)
            nc.vector.tensor_tensor(uv[:, 0:MS, :], uv[:, 0:MS, :],
                                    sbb[:, 0:MS, :], MULT)
            nc.gpsimd.tensor_tensor(uv[:, MS:NC1, :], uv[:, MS:NC1, :],
                                    sbb[:, MS:NC1, :], MULT)
            q1v = q1[:].rearrange("p (c t) -> p c t", c=NC1)
            nc.gpsimd.tensor_scalar(
                q1v[:, :, j * TH:(j + 1) * TH], uv, C_MAGIC, C_MAGIC, ADD, SUB)
            return M1

        def gemm1(t1, q1, rjs):
            """GEMM1 + fused relu^2 evac into per-half r tiles rjs[j]."""
            for op_ in range(NC2 // 2):
                acc = ps_g1.tile([128, 512], F32, tag="a1")
                for half in range(2):
                    oc = 2 * op_ + half
                    for cp in range(NC1):
                        nc.tensor.matmul(
                            acc[:, half * TB:(half + 1) * TB],
                            t1[:, cp * I + oc * 128: cp * I + (oc + 1) * 128],
                            q1[:, cp * TB:(cp + 1) * TB],
                            start=(cp == 0), stop=(cp == NC1 - 1))
                av = acc[:].rearrange("p (o t) -> p o t", o=2)
                for j in range(2):
                    rv = rjs[j][:].rearrange("p (m t) -> p m t", m=NC2)
                    sl = av[:, :, j * TH:(j + 1) * TH]
                    dst = rv[:, 2 * op_:2 * op_ + 2, :]
                    nc.scalar.activation(dst, sl, AF.Relu, bias=0.0)
                    nc.scalar.activation(dst, dst, AF.Square, bias=0.0)

        def h128_2(rj):
            for g in range(NC2 // 4):
                ps = ps_m.tile([128, 512], F32, tag="pm")
                for k in range(4):
                    m = 4 * g + k
                    nc.tensor.matmul(ps[:, k * TH:(k + 1) * TH], h128[:],
                                     rj[:, m * TH:(m + 1) * TH],
                                     start=True, stop=True)
                nc.scalar.copy(rj[:, g * 512:(g + 1) * 512], ps[:])

        def bf2_p2(rj, j):
            for sg in (1, 2, 4, 8, 16):
                _bfly_ip(rj, NC2, sg, TH)
            P2 = scl.tile([128, TH], F32, tag="p2", name=f"p2_{j}")
            nc.vector.tensor_reduce(
                P2[:], rj[:].rearrange("p (m t) -> p t m", m=NC2),
                AX, MAX, apply_absolute_value=True)
            return P2

        def scale_chain(sW, P2, M1, j):
            # cc = (M1*sW0)^2 / (127^2 * 64)
            cc = tiny.tile([128, 1], F32, tag="cc")
            nc.vector.tensor_tensor(cc[:], M1[:], sW[:, 0:1], MULT)
            nc.vector.tensor_tensor(cc[:], cc[:], cc[:], MULT)
            nc.vector.tensor_scalar(cc[:], cc[:], 1.0 / (127.0 * 127.0 * 64.0),
                                    None, MULT)
            tps = ps_m.tile([128, 128], F32, tag="pm")
            nc.tensor.transpose(tps[:], P2[:], ident[:])
            M2 = tiny.tile([128, 1], F32, tag="m2")
            nc.vector.tensor_reduce(M2[:], tps[:], AX, MAX,
                                    apply_absolute_value=True)
            nc.vector.tensor_tensor(M2[:], M2[:], cc[:], MULT)
            nc.vector.tensor_scalar(M2[:], M2[:], 1e-5, None, MAX)
            s2t = tiny.tile([128, 1], F32, tag="s2t")
            nc.vector.reciprocal(s2t[:], M2[:])
            nc.vector.tensor_tensor(s2t[:], s2t[:], cc[:], MULT)
            nc.vector.tensor_scalar(s2t[:], s2t[:], 127.0, None, MULT)
            fb = tiny.tile([128, 1], F32, tag=f"fb{j}")
            nc.vector.tensor_tensor(fb[:], M2[:], sW[:, 1:2], MULT)
            nc.vector.tensor_scalar(fb[:], fb[:], 1.0 / 127.0, None, MULT)
            s2b = col_scales_bcast(s2t)
            return s2b, fb

        def mult_round(rj, s2b, q2):
            rv = rj[:].rearrange("p (m t) -> p m t", m=NC2)
            sbb = s2b[:, None, :].broadcast_to([128, NC2, TH])
            MS = int(NC2 * MSF)
            nc.vector.tensor_tensor(rv[:, 0:MS, :], rv[:, 0:MS, :],
                                    sbb[:, 0:MS, :], MULT)
            nc.gpsimd.tensor_tensor(rv[:, MS:NC2, :], rv[:, MS:NC2, :],
                                    sbb[:, MS:NC2, :], MULT)
            nc.gpsimd.tensor_scalar(q2[:], rj[:], C_MAGIC, C_MAGIC, ADD, SUB)

        def gemm2(b, j, q2, fb):
            acc2s = [ps_g2.tile([128, 512], F32, tag="a2", name=f"a2_{hs}")
                     for hs in range(4)]
            for e in range(NC2 // 2):
                st = slab.tile([128, 2 * H], FP8, tag="t2sl")
                nc.sync.dma_start(
                    st[:].rearrange("p (m h) -> p m h", m=2),
                    t2g[e * 256:(e + 1) * 256, :]
                    .rearrange("(m p) h -> p m h", p=128))
                for hs in range(4):
                    for mi in range(2):
                        m2 = e * 2 + mi
                        nc.tensor.matmul(
                            acc2s[hs][:],
                            q2[:, m2 * TH:(m2 + 1) * TH],
                            st[:, mi * H + hs * 512: mi * H + (hs + 1) * 512],
                            start=(m2 == 0), stop=(m2 == NC2 - 1))
            for hs in range(4):
                ot = otp.tile([128, 512], F32, tag="ot")
                nc.scalar.activation(ot[:], acc2s[hs][:], AF.Identity,
                                     bias=0.0, scale=fb[:])
                nc.sync.dma_start(
                    out_d[b * TB + j * TH: b * TB + (j + 1) * TH,
                          hs * 512:(hs + 1) * 512], ot[:])

        # ------------- emission schedule -------------
        # Prologue: x-path for blocks 0..2 overlaps the scale AllGather and
        # the weight ternarize/AllGather latency; gemm1(0) fires as soon as
        # t1 lands.
        q1s = {}
        rs = {}
        m1s = {}
        q2s = {}
        fbs = {}
        q1s[0] = q1p.tile([128, NC1 * TB], BF16, tag="q1", name="q1_0")
        m1_00 = x_half(0, 0, q1s[0])
        sW, rW = finish_scales()
        m1s[0] = [m1_00, x_half(0, 1, q1s[0])]
        for b in range(1, 3):
            q1s[b] = q1p.tile([128, NC1 * TB], BF16, tag="q1", name=f"q1_{b}")
            m1s[b] = [x_half(b, j, q1s[b]) for j in range(2)]

        t1 = weights_phase2(sW, rW)

        # iteration k PE stream:
        #   gemm2(k-2) | scale-bcasts(k-1) | gemm1(k) | h128_2(k) | xA(k+3)
        # iteration k DVE stream:
        #   bf2+P2(k-1) | mult+round(k-1) | gemm1-evac(k) | bf1+quant1(k+3)
        for k in range(NB + 2):
            if 0 <= k - 2 < NB:
                for j in range(2):
                    gemm2(k - 2, j, q2s[(k - 2, j)], fbs[(k - 2, j)])
                    del q2s[(k - 2, j)], fbs[(k - 2, j)]
            if 0 <= k - 1 < NB:
                P2s = [bf2_p2(rs[k - 1][j], j) for j in range(2)]
                sbs = [scale_chain(sW, P2s[j], m1s[k - 1][j], j)
                       for j in range(2)]
                for j in range(2):
                    q2 = q2p.tile([128, NC2 * TH], BF16, tag="q2",
                                  name=f"q2_{k - 1}_{j}")
                    s2b, fb = sbs[j]
                    mult_round(rs[k - 1][j], s2b, q2)
                    q2s[(k - 1, j)] = q2
                    fbs[(k - 1, j)] = fb
                del rs[k - 1], m1s[k - 1]
            if k < NB:
                rjs = [rp.tile([128, NC2 * TH], F32, tag="rj",
                               name=f"r_{k}_{j}") for j in range(2)]
                gemm1(t1, q1s[k], rjs)
                del q1s[k]
                h128_2(rjs[0])
                h128_2(rjs[1])
                rs[k] = rjs
            if 3 <= k + 3 < NB:
                b = k + 3
                q1s[b] = q1p.tile([128, NC1 * TB], BF16, tag="q1",
                                  name=f"q1_{b}")
                m1s[b] = [x_half(b, j, q1s[b]) for j in range(2)]

    nc.finalize()
    return nc


_NC_CACHE = None


def _get_nc():
    global _NC_CACHE
    if _NC_CACHE is None:
        _NC_CACHE = build()
    return _NC_CACHE


def _hadamard128():
    h = np.array([[1.0]], dtype=np.float32)
    while h.shape[0] < 128:
        h = np.block([[h, h], [h, -h]])
    return h.astype(np.float32)


def kernel(hidden_states, w_up, w_down):
    x = np.ascontiguousarray(hidden_states.reshape(TOKENS, H), dtype=np.float32)
    wuT = np.ascontiguousarray(w_up.T, dtype=np.float32)
    wdT = np.ascontiguousarray(w_down.T, dtype=np.float32)
    h128 = _hadamard128()

    nc = _get_nc()
    in_maps = []
    for c in range(NCORES):
        in_maps.append({
            "x": x[c * T:(c + 1) * T],
            "wu_s": np.ascontiguousarray(
                wuT[c * (H // NCORES):(c + 1) * (H // NCORES)]),
            "wd_s": np.ascontiguousarray(
                wdT[c * (I // NCORES):(c + 1) * (I // NCORES)]),
            "h128": h128,
        })
    res = run_bass_kernel_spmd(nc, in_maps, list(range(NCORES))).results
    out = np.concatenate(
        [np.asarray(res[c]["out"], dtype=np.float32) for c in range(NCORES)], axis=0
    )
    return out.reshape(B, S, H)


# revision 22
# speedup vs baseline: 1.4768x; 1.4768x over previous
"""BitFeedForward (Hadamard + int8 act-quant + ternary weights) on 8 TRN2 cores.

v3 — data-parallel over tokens (8192 -> 1024/core), restructured from v2:
  - weight ternarization sharded 8x across cores (each core ternarizes 1/8
    of w_up and w_down), then fp8 AllGather; t1 resident in SBUF, t2
    streamed once per block from the gathered DRAM buffer.
  - per-token absmax via PE-transpose + free-dim reduce + K=1 matmul
    broadcast (no gpsimd partition_all_reduce on the hot path).
  - plain H128 matmul stages (single stationary operand) + in-place
    DVE/GPSIMD butterflies for all inter-chunk FWHT stages.
  - relu^2 evacuation fused into one DVE scalar_tensor_tensor per slice.
  - 2-deep software-pipelined emission so the PE stream
    [gemm2(k-2) | xA(k+1) | gemm1(k) | h128_2(k)] never waits on the DVE
    butterfly chain.
"""
import math
import numpy as np
from contextlib import ExitStack

import concourse.bass as bass
from concourse import bacc
import concourse.tile as tile
import concourse.mybir as mybir
from concourse.bass_utils import run_bass_kernel_spmd
from concourse.masks import make_identity

F32 = mybir.dt.float32
BF16 = mybir.dt.bfloat16
FP8 = mybir.dt.float8e4

NCORES = 8
B, S, H, I = 4, 2048, 2048, 4096
TOKENS = B * S
T = TOKENS // NCORES
TB = 256
NB = T // TB
TH = 128
NC1 = H // 128
NC2 = I // 128
C_MAGIC = 12582912.0
ISQ1 = 1.0 / math.sqrt(H)
WCOUNT = float(H * I)
FR1 = 0.82  # DVE share of FWHT1 butterflies
FR2 = 0.80  # DVE share of FWHT2 butterflies
MSF = 0.62  # DVE share of the scale-multiplies

ADD = mybir.AluOpType.add
SUB = mybir.AluOpType.subtract
MULT = mybir.AluOpType.mult
MAX = mybir.AluOpType.max
MIN = mybir.AluOpType.min
BYPASS = mybir.AluOpType.bypass
AF = mybir.ActivationFunctionType
AX = mybir.AxisListType.X


def _bfly_rng(eng, t, c0, nch, sigma, span, is_gp=False):
    """In-place butterfly over chunks [c0, c0+nch) of t: a'=a+b ; b'=a'-2b."""
    v = (t[:, c0 * span:(c0 + nch) * span]
         .rearrange("p (g two s) -> p g two s", two=2, s=sigma * span))
    a = v[:, :, 0, :]
    bv = v[:, :, 1, :]
    eng.tensor_tensor(a, a, bv, ADD)
    if is_gp:
        eng.tensor_tensor(bv, bv, bv, ADD)
        eng.tensor_tensor(bv, a, bv, SUB)
    else:
        eng.scalar_tensor_tensor(bv, bv, -2.0, a, MULT, ADD)


def _fwht_chunks(nc, t, nchunk, span, n_full):
    """FWHT over the chunk axis, stages in DECREASING sigma (stages commute).

    The first n_full stages run whole-width on DVE; after them the chunk
    axis splits into 2^n_full independent groups — the last group's
    remaining stages run on GPSIMD, the rest on DVE (sync-free chains).
    """
    sigma = nchunk // 2
    for _ in range(n_full):
        _bfly_rng(nc.vector, t, 0, nchunk, sigma, span)
        sigma //= 2
    grp = nchunk >> n_full
    ngrp = 1 << n_full
    while sigma >= 1:
        s = sigma
        _bfly_rng(nc.gpsimd, t, nchunk - grp, grp, s, span, is_gp=True)
        for g in range(ngrp - 1):
            _bfly_rng(nc.vector, t, g * grp, grp, s, span)
        sigma //= 2


def build():
    nc = bacc.Bacc()
    x_in = nc.declare_dram_parameter("x", [T, H], F32, isOutput=False)
    wus_in = nc.declare_dram_parameter("wu_s", [H // NCORES, I], F32, isOutput=False)
    wds_in = nc.declare_dram_parameter("wd_s", [I // NCORES, H], F32, isOutput=False)
    h128_in = nc.declare_dram_parameter("h128", [128, 128], F32, isOutput=False)
    out_d = nc.declare_dram_parameter("out", [T, H], F32, isOutput=True)

    RG = [list(range(NCORES))]

    with tile.TileContext(nc) as tc, ExitStack() as ctx:
        const = ctx.enter_context(tc.tile_pool(name="const", bufs=1))
        wq = ctx.enter_context(tc.tile_pool(name="wq", bufs=2))
        tsb = ctx.enter_context(tc.tile_pool(name="tsb", bufs=2))
        t1p = ctx.enter_context(tc.tile_pool(name="t1", bufs=1))
        slab = ctx.enter_context(tc.tile_pool(name="slab", bufs=3))
        xinp = ctx.enter_context(tc.tile_pool(name="xinp", bufs=3))
        fw1 = ctx.enter_context(tc.tile_pool(name="fw1", bufs=2))
        q1p = ctx.enter_context(tc.tile_pool(name="q1", bufs=3))
        rp = ctx.enter_context(tc.tile_pool(name="rp", bufs=2))
        q2p = ctx.enter_context(tc.tile_pool(name="q2", bufs=2))
        otp = ctx.enter_context(tc.tile_pool(name="otp", bufs=2))
        scl = ctx.enter_context(tc.tile_pool(name="scl", bufs=2))
        tiny = ctx.enter_context(tc.tile_pool(name="tiny", bufs=6))
        dram = ctx.enter_context(tc.tile_pool(name="dram", bufs=1, space="DRAM"))
        ps_m = ctx.enter_context(tc.tile_pool(name="ps_m", bufs=2, space="PSUM"))
        ps_g1 = ctx.enter_context(tc.tile_pool(name="ps_g1", bufs=2, space="PSUM"))
        ps_g2 = ctx.enter_context(tc.tile_pool(name="ps_g2", bufs=4, space="PSUM"))

        ident = const.tile([128, 128], F32)
        make_identity(nc, ident[:])
        h128 = const.tile([128, 128], F32)
        nc.sync.dma_start(h128[:], h128_in[:])
        ones_col = const.tile([128, 1], F32)
        nc.vector.memset(ones_col[:], 1.0)
        ones_row = const.tile([1, 128], F32)
        nc.vector.memset(ones_row[:], 1.0)
        biasC = const.tile([128, 1], F32)
        nc.vector.memset(biasC[:], C_MAGIC)
        biasNC = const.tile([128, 1], F32)
        nc.vector.memset(biasNC[:], -C_MAGIC)

        # ------------- weight scale partials + AllReduce -------------
        def wchunks(src, nrow, width):
            """Yield ([128,2048] slice-view, row, colhalf) subchunks."""
            for i in range(nrow):
                for hc in range(width // 2048):
                    yield (src[i * 128:(i + 1) * 128,
                               hc * 2048:(hc + 1) * 2048], i, hc)

        def abs_total(src, nrow, width, tagp):
            parts = []
            for sl, i, hc in wchunks(src, nrow, width):
                ch = wq.tile([128, 2048], F32, tag="wch", name=f"w_{tagp}_{i}_{hc}")
                nc.gpsimd.dma_start(ch[:], sl)
                acc = tiny.tile([128, 1], F32, tag="wacc")
                nc.vector.tensor_reduce(acc[:], ch[:], AX, ADD,
                                        apply_absolute_value=True)
                parts.append(acc)
            tot = tiny.tile([128, 1], F32, tag=f"{tagp}tot")
            nc.vector.tensor_add(tot[:], parts[0][:], parts[1][:])
            for a in parts[2:]:
                nc.vector.tensor_add(tot[:], tot[:], a[:])
            return tot

        pu = abs_total(wus_in, 2, I, "au")
        pd = abs_total(wds_in, 4, H, "ad")
        psum2 = ps_m.tile([1, 2], F32, tag="pm")
        nc.tensor.matmul(psum2[:, 0:1], pu[:], ones_col[:], start=True, stop=True)
        nc.tensor.matmul(psum2[:, 1:2], pd[:], ones_col[:], start=True, stop=True)
        part = tiny.tile([1, 2], F32)
        nc.vector.tensor_copy(part[:], psum2[:])

        ccin = dram.tile([1, 2], F32, tag="ccin")
        ccg = dram.tile([NCORES, 2], F32, tag="ccg", addr_space="Shared")
        nc.gpsimd.dma_start(ccin[:], part[:])
        nc.gpsimd.collective_compute(
            "AllGather", BYPASS, replica_groups=RG,
            ins=[ccin.opt()], outs=[ccg.opt()])

        ones8 = const.tile([8, 128], F32)
        nc.vector.memset(ones8[:], 1.0)

        def finish_scales():
            cc8 = tiny.tile([8, 2], F32)
            nc.gpsimd.dma_start(cc8[:], ccg[:])
            # sum the 8 rank rows AND broadcast to 128 partitions in one mm
            sW_ps = ps_m.tile([128, 2], F32, tag="pm")
            nc.tensor.matmul(sW_ps[:], ones8[:], cc8[:], start=True, stop=True)
            sW = tiny.tile([128, 2], F32)
            nc.vector.tensor_copy(sW[:], sW_ps[:])
            nc.vector.tensor_scalar(sW[:], sW[:], 1.0 / WCOUNT, 1e-5, MULT, MAX)
            rW = tiny.tile([128, 2], F32)
            nc.vector.reciprocal(rW[:], sW[:])
            return sW, rW

        # ------------- sharded ternarize + fp8 AllGathers (t1 first) -------------
        t1sA = dram.tile([H // NCORES, I // 2], FP8, tag="t1sA")
        t1sB = dram.tile([H // NCORES, I // 2], FP8, tag="t1sB")
        t1gA = dram.tile([H, I // 2], FP8, tag="t1gA", addr_space="Shared")
        t1gB = dram.tile([H, I // 2], FP8, tag="t1gB", addr_space="Shared")
        t2s = dram.tile([I // NCORES, H], FP8, tag="t2s")
        t2g = dram.tile([I, H], FP8, tag="t2g", addr_space="Shared")

        def ternarize(rW, src, nrow, width, col, tagp, dst_of):
            for sidx, (sl, i, hc) in enumerate(wchunks(src, nrow, width)):
                ch = wq.tile([128, 2048], F32, tag="wch", name=f"wb_{tagp}_{i}_{hc}")
                nc.sync.dma_start(ch[:], sl)
                ob = tsb.tile([128, 2048], FP8, tag="tsb")
                for q in range(4):
                    tmp = ps_m.tile([128, 512], F32, tag="pm")
                    nc.scalar.activation(tmp[:], ch[:, q * 512:(q + 1) * 512],
                                         AF.Identity, bias=biasC[:],
                                         scale=rW[:, col:col + 1])
                    nc.scalar.activation(ob[:, q * 512:(q + 1) * 512], tmp[:],
                                         AF.Sign, bias=biasNC[:], scale=1.0)
                nc.sync.dma_start(dst_of(i, hc), ob[:])

        def t1_dst(i, hc):
            g = t1sA if hc == 0 else t1sB
            return g[i * 128:(i + 1) * 128, :]

        def t2_dst(i, hc):
            return t2s[i * 128:(i + 1) * 128, :]

        def weights_phase2(sW, rW):
            # t1 I-column half A first: its AllGather (small -> mesh algo)
            # unblocks the first half of GEMM1's output chunks early.
            for hc, shard, gath in ((0, t1sA, t1gA), (1, t1sB, t1gB)):
                for i in range(2):
                    sl = wus_in[i * 128:(i + 1) * 128,
                                hc * 2048:(hc + 1) * 2048]
                    ch = wq.tile([128, 2048], F32, tag="wch",
                                 name=f"wb_u_{i}_{hc}")
                    nc.sync.dma_start(ch[:], sl)
                    ob = tsb.tile([128, 2048], FP8, tag="tsb")
                    for q in range(4):
                        tmp = ps_m.tile([128, 512], F32, tag="pm")
                        nc.scalar.activation(tmp[:], ch[:, q * 512:(q + 1) * 512],
                                             AF.Identity, bias=biasC[:],
                                             scale=rW[:, 0:1])
                        nc.scalar.activation(ob[:, q * 512:(q + 1) * 512],
                                             tmp[:], AF.Sign, bias=biasNC[:],
                                             scale=1.0)
                    nc.sync.dma_start(shard[i * 128:(i + 1) * 128, :], ob[:])
                nc.gpsimd.collective_compute(
                    "AllGather", BYPASS, replica_groups=RG,
                    ins=[shard.opt()], outs=[gath.opt()])
            ternarize(rW, wds_in, 4, H, 1, "d", t2_dst)
            nc.gpsimd.collective_compute(
                "AllGather", BYPASS, replica_groups=RG,
                ins=[t2s.opt()], outs=[t2g.opt()])
            t1 = t1p.tile([128, NC1 * I], FP8)  # resident ternary w_up^T
            for kc in range(NC1):
                nc.sync.dma_start(t1[:, kc * I:kc * I + 2048],
                                  t1gA[kc * 128:(kc + 1) * 128, :])
            for kc in range(NC1):
                nc.sync.dma_start(t1[:, kc * I + 2048:(kc + 1) * I],
                                  t1gB[kc * 128:(kc + 1) * 128, :])
            return t1

        # ------------- per-token scale helpers -------------
        def col_scales_bcast(vec):
            """[128,1] token-on-partition vector -> [128, TH] SBUF broadcast."""
            rps = ps_m.tile([1, 128], F32, tag="pm")
            nc.tensor.transpose(rps[:], vec[:], ident[:])
            row = tiny.tile([1, 128], F32, tag="row")
            nc.vector.tensor_copy(row[:], rps[:])
            bps = ps_m.tile([128, 128], F32, tag="pm")
            nc.tensor.matmul(bps[:], ones_row[:], row[:], start=True, stop=True)
            sb = scl.tile([128, TH], F32, tag="bc")
            nc.scalar.copy(sb[:], bps[:])
            return sb

        def x_half(b, j, q1):
            """Transpose + FWHT + int8 quant for token-half j of block b.

            Returns the per-token M1 vector (kept for layer-2 scales)."""
            tok0 = b * TB + j * TH
            u = fw1.tile([128, NC1 * TH], F32, tag="fw1")
            for g in range(4):
                xs = xinp.tile([128, 512], F32, tag="xin")
                nc.sync.dma_start(
                    xs[:], x_in[tok0:tok0 + TH, g * 512:(g + 1) * 512])
                pt = ps_m.tile([128, 512], F32, tag="pm")
                for k in range(4):
                    nc.tensor.transpose(
                        pt[:, k * 128:(k + 1) * 128],
                        xs[:, k * 128:(k + 1) * 128], ident[:])
                nc.scalar.copy(u[:, g * 512:(g + 1) * 512], pt[:])
            for g in range(4):
                u1 = ps_m.tile([128, 512], F32, tag="pm")
                for k in range(4):
                    c = 4 * g + k
                    nc.tensor.matmul(u1[:, k * TH:(k + 1) * TH], h128[:],
                                     u[:, c * TH:(c + 1) * TH],
                                     start=True, stop=True)
                nc.scalar.copy(u[:, g * 512:(g + 1) * 512], u1[:])
            _fwht_chunks(nc, u, NC1, TH, 2)
            # per-token absmax over [partitions x chunks]
            P1 = scl.tile([128, TH], F32, tag="p1")
            nc.vector.tensor_reduce(
                P1[:], u[:].rearrange("p (c t) -> p t c", c=NC1),
                AX, MAX, apply_absolute_value=True)
            tps = ps_m.tile([128, 128], F32, tag="pm")
            nc.tensor.transpose(tps[:], P1[:], ident[:])
            M1 = tiny.tile([128, 1], F32, tag="m1")
            nc.vector.tensor_reduce(M1[:], tps[:], AX, MAX,
                                    apply_absolute_value=True)
            nc.vector.tensor_scalar(M1[:], M1[:], ISQ1, 1e-5, MULT, MAX)
            s1t = tiny.tile([128, 1], F32, tag="s1t")
            nc.vector.reciprocal(s1t[:], M1[:])
            nc.vector.tensor_scalar(s1t[:], s1t[:], 127.0 * ISQ1, None, MULT)
            s1b = col_scales_bcast(s1t)
            uv = u[:].rearrange("p (c t) -> p c t", c=NC1)
            sbb = s1b[:, None, :].broadcast_to([128, NC1, TH])
            MS = int(NC1 * MSF)
            nc.vector.tensor_tensor(uv[:, 0:MS, :], uv[:, 0:MS, :],
                                    sbb[:, 0:MS, :], MULT)
            nc.gpsimd.tensor_tensor(uv[:, MS:NC1, :], uv[:, MS:NC1, :],
                                    sbb[:, MS:NC1, :], MULT)
            q1v = q1[:].rearrange("p (c t) -> p c t", c=NC1)
            nc.vector.tensor_scalar(
                q1v[:, :, j * TH:(j + 1) * TH], uv, C_MAGIC, C_MAGIC, ADD, SUB)
            return M1

        def gemm1(t1, q1, rjs):
            """GEMM1 + fused relu^2 evac into per-half r tiles rjs[j]."""
            for op_ in range(NC2 // 2):
                acc = ps_g1.tile([128, 512], F32, tag="a1")
                for half in range(2):
                    oc = 2 * op_ + half
                    for cp in range(NC1):
                        nc.tensor.matmul(
                            acc[:, half * TB:(half + 1) * TB],
                            t1[:, cp * I + oc * 128: cp * I + (oc + 1) * 128],
                            q1[:, cp * TB:(cp + 1) * TB],
                            start=(cp == 0), stop=(cp == NC1 - 1))
                av = acc[:].rearrange("p (o t) -> p o t", o=2)
                for j in range(2):
                    rv = rjs[j][:].rearrange("p (m t) -> p m t", m=NC2)
                    sl = av[:, :, j * TH:(j + 1) * TH]
                    dst = rv[:, 2 * op_:2 * op_ + 2, :]
                    nc.scalar.activation(dst, sl, AF.Relu, bias=0.0)
                    nc.scalar.activation(dst, dst, AF.Square, bias=0.0)

        def h128_2(rj):
            for g in range(NC2 // 4):
                ps = ps_m.tile([128, 512], F32, tag="pm")
                for k in range(4):
                    m = 4 * g + k
                    nc.tensor.matmul(ps[:, k * TH:(k + 1) * TH], h128[:],
                                     rj[:, m * TH:(m + 1) * TH],
                                     start=True, stop=True)
                nc.scalar.copy(rj[:, g * 512:(g + 1) * 512], ps[:])

        def bf2_p2(rj, j):
            _fwht_chunks(nc, rj, NC2, TH, 2)
            P2 = scl.tile([128, TH], F32, tag="p2", name=f"p2_{j}")
            nc.vector.tensor_reduce(
                P2[:], rj[:].rearrange("p (m t) -> p t m", m=NC2),
                AX, MAX, apply_absolute_value=True)
            return P2

        def scale_chain(sW, P2, M1, j):
            # cc = (M1*sW0)^2 / (127^2 * 64)
            cc = tiny.tile([128, 1], F32, tag="cc")
            nc.vector.tensor_tensor(cc[:], M1[:], sW[:, 0:1], MULT)
            nc.vector.tensor_tensor(cc[:], cc[:], cc[:], MULT)
            nc.vector.tensor_scalar(cc[:], cc[:], 1.0 / (127.0 * 127.0 * 64.0),
                                    None, MULT)
            tps = ps_m.tile([128, 128], F32, tag="pm")
            nc.tensor.transpose(tps[:], P2[:], ident[:])
            M2 = tiny.tile([128, 1], F32, tag="m2")
            nc.vector.tensor_reduce(M2[:], tps[:], AX, MAX,
                                    apply_absolute_value=True)
            nc.vector.tensor_tensor(M2[:], M2[:], cc[:], MULT)
            nc.vector.tensor_scalar(M2[:], M2[:], 1e-5, None, MAX)
            s2t = tiny.tile([128, 1], F32, tag="s2t")
            nc.vector.reciprocal(s2t[:], M2[:])
            nc.vector.tensor_tensor(s2t[:], s2t[:], cc[:], MULT)
            nc.vector.tensor_scalar(s2t[:], s2t[:], 127.0, None, MULT)
            fb = tiny.tile([128, 1], F32, tag=f"fb{j}")
            nc.vector.tensor_tensor(fb[:], M2[:], sW[:, 1:2], MULT)
            nc.vector.tensor_scalar(fb[:], fb[:], 1.0 / 127.0, None, MULT)
            s2b = col_scales_bcast(s2t)
            return s2b, fb

        def mult_round(rj, s2b, q2):
            rv = rj[:].rearrange("p (m t) -> p m t", m=NC2)
            sbb = s2b[:, None, :].broadcast_to([128, NC2, TH])
            MS = int(NC2 * MSF)
            nc.vector.tensor_tensor(rv[:, 0:MS, :], rv[:, 0:MS, :],
                                    sbb[:, 0:MS, :], MULT)
            nc.gpsimd.tensor_tensor(rv[:, MS:NC2, :], rv[:, MS:NC2, :],
                                    sbb[:, MS:NC2, :], MULT)
            nc.vector.tensor_scalar(q2[:], rj[:], C_MAGIC, C_MAGIC, ADD, SUB)

        def gemm2(b, j, q2, fb):
            acc2s = [ps_g2.tile([128, 512], F32, tag="a2", name=f"a2_{hs}")
                     for hs in range(4)]
            for e in range(NC2 // 2):
                st = slab.tile([128, 2 * H], FP8, tag="t2sl")
                nc.sync.dma_start(
                    st[:].rearrange("p (m h) -> p m h", m=2),
                    t2g[e * 256:(e + 1) * 256, :]
                    .rearrange("(m p) h -> p m h", p=128))
                for hs in range(4):
                    for mi in range(2):
                        m2 = e * 2 + mi
                        nc.tensor.matmul(
                            acc2s[hs][:],
                            q2[:, m2 * TH:(m2 + 1) * TH],
                            st[:, mi * H + hs * 512: mi * H + (hs + 1) * 512],
                            start=(m2 == 0), stop=(m2 == NC2 - 1))
            for hs in range(4):
                ot = otp.tile([128, 512], F32, tag="ot")
                nc.scalar.activation(ot[:], acc2s[hs][:], AF.Identity,
                                     bias=0.0, scale=fb[:])
                nc.sync.dma_start(
                    out_d[b * TB + j * TH: b * TB + (j + 1) * TH,
                          hs * 512:(hs + 1) * 512], ot[:])

        # ------------- emission schedule -------------
        # Prologue: x-path for blocks 0..2 overlaps the scale AllGather and
        # the weight ternarize/AllGather latency; gemm1(0) fires as soon as
        # t1 lands.
        q1s = {}
        rs = {}
        m1s = {}
        q2s = {}
        fbs = {}
        q1s[0] = q1p.tile([128, NC1 * TB], BF16, tag="q1", name="q1_0")
        m1_00 = x_half(0, 0, q1s[0])
        sW, rW = finish_scales()
        m1s[0] = [m1_00, x_half(0, 1, q1s[0])]
        for b in range(1, 3):
            q1s[b] = q1p.tile([128, NC1 * TB], BF16, tag="q1", name=f"q1_{b}")
            m1s[b] = [x_half(b, j, q1s[b]) for j in range(2)]

        t1 = weights_phase2(sW, rW)

        # iteration k PE stream:
        #   gemm2(k-2) | scale-bcasts(k-1) | gemm1(k) | h128_2(k) | xA(k+3)
        # iteration k DVE stream:
        #   bf2+P2(k-1) | mult+round(k-1) | gemm1-evac(k) | bf1+quant1(k+3)
        for k in range(NB + 2):
            if 0 <= k - 2 < NB:
                for j in range(2):
                    gemm2(k - 2, j, q2s[(k - 2, j)], fbs[(k - 2, j)])
                    del q2s[(k - 2, j)], fbs[(k - 2, j)]
            if 0 <= k - 1 < NB:
                P2s = [bf2_p2(rs[k - 1][j], j) for j in range(2)]
                sbs = [scale_chain(sW, P2s[j], m1s[k - 1][j], j)
                       for j in range(2)]
                for j in range(2):
                    q2 = q2p.tile([128, NC2 * TH], BF16, tag="q2",
                                  name=f"q2_{k - 1}_{j}")
                    s2b, fb = sbs[j]
                    mult_round(rs[k - 1][j], s2b, q2)
                    q2s[(k - 1, j)] = q2
                    fbs[(k - 1, j)] = fb
                del rs[k - 1], m1s[k - 1]
            if k < NB:
                rjs = [rp.tile([128, NC2 * TH], F32, tag="rj",
                               name=f"r_{k}_{j}") for j in range(2)]
                gemm1(t1, q1s[k], rjs)
                del q1s[k]
                h128_2(rjs[0])
                h128_2(rjs[1])
                rs[k] = rjs
            if 3 <= k + 3 < NB:
                b = k + 3
                q1s[b] = q1p.tile([128, NC1 * TB], BF16, tag="q1",
                                  name=f"q1_{b}")
                m1s[b] = [x_half(b, j, q1s[b]) for j in range(2)]

    nc.finalize()
    return nc


_NC_CACHE = None


def _get_nc():
    global _NC_CACHE
    if _NC_CACHE is None:
        _NC_CACHE = build()
    return _NC_CACHE


def _hadamard128():
    h = np.array([[1.0]], dtype=np.float32)
    while h.shape[0] < 128:
        h = np.block([[h, h], [h, -h]])
    return h.astype(np.float32)


def kernel(hidden_states, w_up, w_down):
    x = np.ascontiguousarray(hidden_states.reshape(TOKENS, H), dtype=np.float32)
    wuT = np.ascontiguousarray(w_up.T, dtype=np.float32)
    wdT = np.ascontiguousarray(w_down.T, dtype=np.float32)
    h128 = _hadamard128()

    nc = _get_nc()
    in_maps = []
    for c in range(NCORES):
        in_maps.append({
            "x": x[c * T:(c + 1) * T],
            "wu_s": np.ascontiguousarray(
                wuT[c * (H // NCORES):(c + 1) * (H // NCORES)]),
            "wd_s": np.ascontiguousarray(
                wdT[c * (I // NCORES):(c + 1) * (I // NCORES)]),
            "h128": h128,
        })
    res = run_bass_kernel_spmd(nc, in_maps, list(range(NCORES))).results
    out = np.concatenate(
        [np.asarray(res[c]["out"], dtype=np.float32) for c in range(NCORES)], axis=0
    )
    return out.reshape(B, S, H)
